# revision 1
# baseline (speedup 1.0000x reference)
"""
Trainium2 Bass kernel for nn_BMM_S8T_S8N_S8T:
  y[b,m,n] = sat_i8(round(alpha * sum_k a[b,m,k] * b[b,n,k]))
with a,b int8 [128, 1024, 128], alpha scalar.

Strategy (8 NeuronCores, batch-parallel, 16 batches/core):
 - Host: XOR inputs with 0x80 (biased-uint8 encoding). On-chip transposes move
   raw bytes; the evacuation subtracts 128 to recover signed values exactly.
 - Per 128x128 input tile: PE transpose in fp8e4-bitcast mode (pure byte
   permutation, stride-2 PSUM output with zero high bytes). All 16 tiles of a
   batch share one 2-bank PSUM tile.
 - Evacuate with one u16 PSUM->SBUF copy (2x DVE mode) + one subtract-128
   u16->bf16 op (4x DVE mode) per batch.
 - Main matmuls in bf16 with fp32 PSUM accumulation: bit-exact for int8 data
   (|acc| <= 2^21 < 2^24).
 - Epilogue: single tensor_scalar/activation op per [128,1024] PSUM tile:
   int8 out = rne_sat(alpha*acc) - matches jnp.clip(jnp.round(...)) bit-exact.
 - DMA spread across both HWDGE queues (sync + scalar), ~200 GB/s each.
"""

import sys

sys.path.insert(0, "/opt/trn_rl_repo")

import numpy as np

N_CORES = 8
B, M, N, K = 128, 1024, 1024, 128
BPC = B // N_CORES  # batches per core
MT = M // 128
NT = N // 128
TT = MT + NT  # transposed tiles per batch

_cache = {}


def _build(alpha: float):
    import concourse.bacc as bacc
    import concourse.tile as tile
    import concourse.mybir as mybir
    from concourse.masks import make_identity

    nc = bacc.Bacc("TRN2", target_bir_lowering=False, debug=False)

    a_x = nc.dram_tensor("a_x", [BPC, M, K], mybir.dt.int8, kind="ExternalInput")
    b_x = nc.dram_tensor("b_x", [BPC, N, K], mybir.dt.int8, kind="ExternalInput")
    y = nc.dram_tensor("y", [BPC, M, N], mybir.dt.int8, kind="ExternalOutput")

    fp8 = mybir.dt.float8e4
    u16 = mybir.dt.uint16
    bf16 = mybir.dt.bfloat16
    f32 = mybir.dt.float32
    i8 = mybir.dt.int8

    with tile.TileContext(nc) as tc:
        with (
            tc.tile_pool(name="const", bufs=1) as cpool,
            tc.tile_pool(name="inp", bufs=6) as ipool,
            tc.tile_pool(name="bfp", bufs=4) as bfpool,
            tc.tile_pool(name="outp", bufs=16) as opool,
            tc.tile_pool(name="pst", bufs=1, space="PSUM") as pst,
            tc.tile_pool(name="psmm", bufs=3, space="PSUM") as psmm,
        ):
            ident = cpool.tile([128, 128], fp8)
            make_identity(nc, ident[:])

            nat = [None] * BPC  # (a_nat, b_nat)
            tps = [None] * BPC  # transpose psum tiles
            bufs = {}  # bi -> ab_bf
            rawd = {}  # bi -> raw u16 staging

            def load_batch(bi):
                a_nat = ipool.tile([128, MT, K], i8, tag="a_nat")
                b_nat = ipool.tile([128, NT, K], i8, tag="b_nat")
                nc.sync.dma_start(
                    out=a_nat[:], in_=a_x[bi].rearrange("(t p) k -> p t k", p=128)
                )
                nc.gpsimd.dma_start(
                    out=b_nat[:], in_=b_x[bi].rearrange("(t p) k -> p t k", p=128)
                )
                nat[bi] = (a_nat, b_nat)

            def transpose_slice(bi, lo, hi):
                # transpose tiles [lo, hi) of the combined 16-tile space
                # (tiles 0..MT-1 from a, MT..TT-1 from b)
                a_nat, b_nat = nat[bi]
                if lo == 0:
                    ta_new = pst.tile([128, MT, 128, 2], fp8, tag="ta")
                    tb_new = pst.tile([128, NT, 128, 2], fp8, tag="tb")
                    tps[bi] = (ta_new, tb_new)
                ta, tb = tps[bi]
                for t in range(lo, hi):
                    if t < MT:
                        nc.tensor.transpose(
                            ta[:, t, :, 0], a_nat[:, t, :].bitcast(fp8), ident[:]
                        )
                    else:
                        nc.tensor.transpose(
                            tb[:, t - MT, :, 0],
                            b_nat[:, t - MT, :].bitcast(fp8),
                            ident[:],
                        )

            def evac_piece(bi, step):
                # 2x copies (steps 0,1) + one 4x subtract (step 2) for 16 tiles
                ta, tb = tps[bi]
                if step == 0:
                    raw = bfpool.tile([128, TT * 128], u16, tag="raw")
                    rawd[bi] = raw
                    nc.vector.tensor_copy(
                        out=raw[:, : MT * 128],
                        in_=ta[:].bitcast(u16).rearrange("p t k o -> p (t k o)"),
                    )
                elif step == 1:
                    nc.vector.tensor_copy(
                        out=rawd[bi][:, MT * 128 :],
                        in_=tb[:].bitcast(u16).rearrange("p t k o -> p (t k o)"),
                    )
                else:
                    ab_bf = bfpool.tile([128, TT * 128], bf16, tag="ab")
                    nc.vector.tensor_scalar(
                        out=ab_bf[:],
                        in0=rawd.pop(bi)[:],
                        scalar1=128.0,
                        scalar2=None,
                        op0=mybir.AluOpType.subtract,
                    )
                    bufs[bi] = ab_bf

            def evac_batch(bi):
                for s in range(3):
                    evac_piece(bi, s)

            # prologue: two-batch lookahead
            load_batch(0)
            load_batch(1)
            transpose_slice(0, 0, TT)
            evac_batch(0)
            transpose_slice(1, 0, TT)

            for bi in range(BPC):
                ab_bf = bufs.pop(bi)
                at_bf = ab_bf[:, : MT * 128]
                bt_bf = ab_bf[:, MT * 128 :]
                if bi + 2 < BPC:
                    load_batch(bi + 2)
                # evac for batch i+1: its transposes ran during batch i-1
                if bi + 1 < BPC:
                    evac_batch(bi + 1)

                for mt in range(MT):
                    mm = psmm.tile([128, N], f32, tag="mm")
                    for nh in range(2):
                        nc.tensor.matmul(
                            mm[:, nh * 512 : (nh + 1) * 512],
                            at_bf[:, mt * 128 : (mt + 1) * 128],
                            bt_bf[:, nh * 512 : (nh + 1) * 512],
                            start=True,
                            stop=True,
                        )
                    # batch i+2's transposes spread over the whole mt loop
                    if bi + 2 < BPC:
                        transpose_slice(bi + 2, 2 * mt, 2 * mt + 2)

                    y_sb = opool.tile([128, N], i8, tag="y")
                    if mt in (1, 5, 7):  # DVE (also does evacs); ACT gets 5
                        nc.vector.tensor_scalar(
                            out=y_sb[:],
                            in0=mm[:],
                            scalar1=float(alpha),
                            scalar2=None,
                            op0=mybir.AluOpType.mult,
                        )
                    else:
                        nc.scalar.activation(
                            out=y_sb[:],
                            in_=mm[:],
                            func=mybir.ActivationFunctionType.Copy,
                            scale=float(alpha),
                        )
                    # stores: sync HWDGE queue and gpsimd SWDGE alternate,
                    # keeping the ACT engine free of store issues
                    store_eng = nc.sync if mt % 2 == 0 else nc.gpsimd
                    store_eng.dma_start(
                        out=y[bi, mt * 128 : (mt + 1) * 128, :], in_=y_sb[:]
                    )

    nc.compile()
    return nc


def _get_nc(alpha: float):
    key = float(alpha)
    if key not in _cache:
        _cache[key] = _build(key)
    return _cache[key]


def kernel(a, b, alpha):
    from concourse.bass_utils import run_bass_kernel_spmd

    a = np.asarray(a)
    b = np.asarray(b)
    assert a.shape == (B, M, K) and a.dtype == np.int8
    assert b.shape == (B, N, K) and b.dtype == np.int8

    nc = _get_nc(float(alpha))

    # biased-uint8 encoding: bytes xor 0x80; kernel subtracts 128 on-chip
    ax = (a.view(np.uint8) ^ 0x80).view(np.int8)
    bx = (b.view(np.uint8) ^ 0x80).view(np.int8)

    in_maps = [
        {
            "a_x": np.ascontiguousarray(ax[c * BPC : (c + 1) * BPC]),
            "b_x": np.ascontiguousarray(bx[c * BPC : (c + 1) * BPC]),
        }
        for c in range(N_CORES)
    ]
    res = run_bass_kernel_spmd(nc, in_maps, list(range(N_CORES)))
    out = np.concatenate([r["y"] for r in res.results], axis=0)
    return out.astype(np.int8)

